# revision 11
# baseline (speedup 1.0000x reference)
"""Trainium2 Bass kernel for nn_ConstantQResonantPacket (B=32768, D=512, K=1024).

psi[b,k] = exp(-dist2(x_b,c_k)/(2*sigma_k^2)) * (ar_k + i*ai_k) * exp(i*(x_b.w_k + phase_k))

Data-parallel over batch across 8 cores; on-chip layout [k partitions, b free].

Key algebra/precision moves:
  * amp -> R*e^{i*phi0}: phi0 folded into the phase offset, R into the envelope.
  * sigma_k = ||w_k||^2 + 1e-4 ~ 4600 -> dist2/(2 sigma^2) <= ~6e-5, so
    R*exp(-a) = R*(1-a) to ~1e-9 relative: the entire envelope is a LINEAR
    function of dist2 and is folded into the centers matmul accumulation:
    PSUM_c = c0 + c1*(x_sq + c_sq - 2 x.c), with c0 = R, c1 = -R/(2 sigma^2).
  * all matmuls run in fp16 at full PE rate; operands are split hi/lo on the
    host and a 3-term split (hi.hi + hi.lo + lo.hi) gives ~fp32-grade phase
    precision (~22 effective mantissa bits) at 1 cycle/row.
  * u = x @ (omega/2pi).T; range reduction via DVE magic-number round
    (w2 = round(u+phi)+M), v_neg = (w2-M)-u; then
    sin = Sin(-2pi*v_neg + 2pi*phi), |.| = Abs(-v_neg + phi),
    cos = Sin(-2pi*|.| + pi/2) -- all inside Sin's [-pi,pi] table domain.
  * real/imag = PSUM_c * cos/sin on DVE; fp32 outputs [K, B_shard];
    host transposes shards into the complex64 (B, K) result.
"""
import numpy as np

import concourse.tile as tile
from concourse import bacc, mybir
from concourse.bass_utils import run_bass_kernel_spmd
from contextlib import ExitStack

F32 = mybir.dt.float32
F16 = mybir.dt.float16
AF = mybir.ActivationFunctionType
OP = mybir.AluOpType

N_CORES = 8
B, D, K = 32768, 512, 1024
B_SH = B // N_CORES          # 4096
BT = 512                     # b tile (free dim)
KT = 128                     # k tile (partition dim)
NB = B_SH // BT              # 8
NK = K // KT                 # 8
ND = D // 128                # 4

MAGIC = float(np.float32(1.5 * 2 ** 23))
TWO_PI = float(np.float32(2.0 * np.pi))
HALF_PI = float(np.float32(np.pi / 2.0))
ENV_SCALE = 2.0 ** 24        # keeps fp16 envelope operands in normal range
ENV_DESCALE = float(np.float32(2.0 ** -24))

DROP_ENV = True   # envelope ~ R: deviation <= ~6e-5 rel, below the fp32
                  # noise floor of the reference itself (measured 6.4e-5).
                  # False computes the full linearized envelope via matmul.

_CACHE = {}
LAST_RESULTS = None


def _build(drop_env):
    nc = bacc.Bacc("TRN2", target_bir_lowering=False, debug=False,
                   num_devices=N_CORES)
    t = nc.alloc_sbuf_tensor("uconst-halfpi", [128, 1], F32)
    nc.gpsimd.memset(t.ap(), HALF_PI)
    nc.const_aps.aps[(F32, HALF_PI)] = t.ap()
    nc.all_engine_barrier()

    x_all = nc.dram_tensor("x_all", (D, 2 * B_SH), F16, kind="ExternalInput").ap()
    w_all = nc.dram_tensor("w_all", (D, 2 * K), F16, kind="ExternalInput").ap()
    small = nc.dram_tensor("small", (128, 3 * NK), F32, kind="ExternalInput").ap()
    if not drop_env:
        cTe = nc.dram_tensor("cTe", (D, K), F16, kind="ExternalInput").ap()
        lhsb = nc.dram_tensor("lhsb", (128, K), F16, kind="ExternalInput").ap()
        rhsb = nc.dram_tensor("rhsb", (128, B_SH), F16, kind="ExternalInput").ap()
    out_r = nc.dram_tensor("out_r", (K, B_SH), F32, kind="ExternalOutput").ap()
    out_i = nc.dram_tensor("out_i", (K, B_SH), F32, kind="ExternalOutput").ap()

    with tile.TileContext(nc) as tc, ExitStack() as ctx:
        par = ctx.enter_context(tc.tile_pool(name="par", bufs=1))
        xt = ctx.enter_context(tc.tile_pool(name="xt", bufs=2))
        ew = ctx.enter_context(tc.tile_pool(name="ew", bufs=3))
        ot = ctx.enter_context(tc.tile_pool(name="ot", bufs=3))
        ps = ctx.enter_context(tc.tile_pool(name="ps", bufs=6 if drop_env else 3, space="PSUM"))

        tsmall = par.tile([128, 3 * NK], F32, tag="small")
        tphi = tsmall[:, 0:NK]
        tphi2 = tsmall[:, NK:2 * NK]
        tc0 = tsmall[:, 2 * NK:3 * NK]
        tw_h, tw_l, tce = [], [], []
        tw_all, tx_all = [], []
        for d in range(ND):
            tw = par.tile([128, 2 * K], F16, tag=f"w{d}")
            tw_all.append(tw)
            tw_h.append(tw[:, 0:K])
            tw_l.append(tw[:, K:2 * K])
            if not drop_env:
                tc_ = par.tile([128, K], F16, tag=f"ce{d}")
                tce.append(tc_)
        # interleave: w chunk d, then b0's x chunk d, so k-tile 0 can start ASAP
        for d in range(ND):
            nc.sync.dma_start(tw_all[d][:], w_all[d * 128:(d + 1) * 128, :])
            xa = xt.tile([128, 2 * BT], F16, tag=f"x{d}")
            nc.sync.dma_start(xa[:], x_all[d * 128:(d + 1) * 128, 0:2 * BT])
            tx_all.append(xa)
            if d == 0:
                nc.sync.dma_start(tsmall[:], small)
        if not drop_env:
            for d in range(ND):
                nc.sync.dma_start(tce[d][:], cTe[d * 128:(d + 1) * 128, :])
            tlb = par.tile([128, K], F16, tag="lhsb")
            nc.sync.dma_start(tlb[:], lhsb)

        for b in range(NB):
            bs = slice(b * BT, (b + 1) * BT)
            if b == 0:
                txa = tx_all
            else:
                txa = []
                for d in range(ND):
                    xa = xt.tile([128, 2 * BT], F16, tag=f"x{d}")
                    nc.sync.dma_start(
                        xa[:], x_all[d * 128:(d + 1) * 128,
                                     2 * b * BT:2 * (b + 1) * BT])
                    txa.append(xa)
            txh = [xa[:, 0:BT] for xa in txa]
            txl = [xa[:, BT:2 * BT] for xa in txa]
            if not drop_env:
                trb = xt.tile([128, BT], F16, tag="rhsb")
                nc.sync.dma_start(trb[:], rhsb[:, bs])

            for k in range(NK):
                ks = slice(k * KT, (k + 1) * KT)
                if not drop_env:
                    psc = ps.tile([KT, BT], F32, tag="psc")
                    for d in range(ND):
                        nc.tensor.matmul(psc[:], tce[d][:, ks], txh[d][:],
                                         start=(d == 0), stop=False)
                    nc.tensor.matmul(psc[:], tlb[:, ks], trb[:],
                                     start=False, stop=True)
                # u = x @ (omega/2pi).T : 3-term f32r split
                psw = ps.tile([KT, BT], F32, tag="psw")
                n_mm = 3 * ND
                i = 0
                for d in range(ND):
                    nc.tensor.matmul(psw[:], tw_h[d][:, ks], txh[d][:],
                                     start=(i == 0), stop=(i == n_mm - 1))
                    i += 1
                    nc.tensor.matmul(psw[:], tw_h[d][:, ks], txl[d][:],
                                     start=False, stop=(i == n_mm - 1))
                    i += 1
                    nc.tensor.matmul(psw[:], tw_l[d][:, ks], txh[d][:],
                                     start=False, stop=(i == n_mm - 1))
                    i += 1
                w2 = ew.tile([KT, BT], F32, tag="w2")
                nc.vector.tensor_scalar(w2[:], psw[:], tphi[:, k:k + 1], MAGIC,
                                        OP.add, OP.add)
                vneg = ew.tile([KT, BT], F32, tag="vneg")
                nc.vector.scalar_tensor_tensor(vneg[:], w2[:], MAGIC, psw[:],
                                               OP.subtract, OP.subtract)
                abst = ew.tile([KT, BT], F32, tag="abst")
                nc.scalar.activation(abst[:], vneg[:], AF.Abs,
                                     bias=tphi[:, k:k + 1], scale=-1.0)
                cost = ew.tile([KT, BT], F32, tag="cost")
                nc.scalar.activation(cost[:], abst[:], AF.Sin,
                                     bias=HALF_PI, scale=-TWO_PI)
                sint = ew.tile([KT, BT], F32, tag="sint")
                nc.scalar.activation(sint[:], vneg[:], AF.Sin,
                                     bias=tphi2[:, k:k + 1], scale=-TWO_PI)
                realt = ot.tile([KT, BT], F32, tag="realt")
                imagt = ot.tile([KT, BT], F32, tag="imagt")
                if not drop_env:
                    nc.vector.scalar_tensor_tensor(realt[:], psc[:], ENV_DESCALE,
                                                   cost[:], OP.mult, OP.mult)
                    nc.vector.scalar_tensor_tensor(imagt[:], psc[:], ENV_DESCALE,
                                                   sint[:], OP.mult, OP.mult)
                else:
                    nc.vector.tensor_scalar_mul(realt[:], cost[:], tc0[:, k:k + 1])
                    nc.vector.tensor_scalar_mul(imagt[:], sint[:], tc0[:, k:k + 1])
                nc.sync.dma_start(out_r[ks, bs], realt[:])
                nc.sync.dma_start(out_i[ks, bs], imagt[:])
    nc.compile()
    return nc


def _host_prep(x, omega, phase, amp_real, amp_imag, centers, drop_env):
    f64 = np.float64
    w64 = omega.astype(f64)
    sigma = (w64 * w64).sum(1) + 1e-4
    inv2s2 = 1.0 / (2.0 * sigma * sigma)
    R = np.hypot(amp_real.astype(f64), amp_imag.astype(f64))
    phi0 = np.arctan2(amp_imag.astype(f64), amp_real.astype(f64))
    c0 = R
    c1 = -R * inv2s2

    wT = np.ascontiguousarray((w64 / (2 * np.pi)).T.astype(np.float32))
    whT = wT.astype(np.float16)
    wlT = (wT - whT.astype(np.float32)).astype(np.float16)

    phi_v = (((phase.astype(f64) + phi0) / (2 * np.pi)) % 1.0).astype(np.float32)
    phi_t = np.ascontiguousarray(phi_v.reshape(NK, 128).T)
    phi2_t = np.ascontiguousarray(
        (phi_v.astype(f64) * (2 * np.pi)).astype(np.float32).reshape(NK, 128).T)

    x32 = x.astype(np.float32)
    xh = x32.astype(np.float16)
    xl = (x32 - xh.astype(np.float32)).astype(np.float16)
    xhT = xh.T  # [D, B]
    xlT = xl.T

    w_all = np.concatenate([whT, wlT], axis=1)  # [D, 2K]
    small = np.zeros((128, 3 * NK), np.float32)
    small[:, 0:NK] = phi_t
    small[:, NK:2 * NK] = phi2_t
    shared = dict(w_all=w_all, small=small)
    if not drop_env:
        c64 = centers.astype(f64)
        c_sq = (c64 * c64).sum(1)
        # all env products carry a 2^24 scale (fp16 range), divided out on DVE.
        bias = (c0 + c1 * c_sq) * (2.0 ** 12)        # lhs 2^12 x rhs ones 2^12
        bias_hi = bias.astype(np.float16)
        bias_lo = (bias - bias_hi.astype(np.float64)).astype(np.float16)
        cTe = np.ascontiguousarray(
            (-2.0 * ENV_SCALE * c1[:, None] * c64).T).astype(np.float32).astype(np.float16)
        lhsb = np.zeros((128, K), np.float16)
        lhsb[0] = bias_hi
        lhsb[1] = bias_lo
        lhsb[2] = (c1 * (2.0 ** 18)).astype(np.float32).astype(np.float16)
        shared.update(cTe=cTe, lhsb=lhsb)
        x_sq = ((x32.astype(f64) ** 2).sum(1) * (2.0 ** 6)).astype(np.float32).astype(np.float16)
    else:
        small[:, 2 * NK:3 * NK] = c0.astype(np.float32).reshape(NK, 128).T

    in_maps = []
    for c in range(N_CORES):
        cs = slice(c * B_SH, (c + 1) * B_SH)
        xa = np.empty((D, 2 * B_SH), np.float16)
        for b in range(NB):
            xa[:, 2 * b * BT:(2 * b + 1) * BT] = xhT[:, c * B_SH + b * BT:c * B_SH + (b + 1) * BT]
            xa[:, (2 * b + 1) * BT:2 * (b + 1) * BT] = xlT[:, c * B_SH + b * BT:c * B_SH + (b + 1) * BT]
        m = dict(shared, x_all=xa)
        if not drop_env:
            rb = np.zeros((128, B_SH), np.float16)
            rb[0] = np.float16(2.0 ** 12)
            rb[1] = np.float16(2.0 ** 12)
            rb[2] = x_sq[cs]
            m["rhsb"] = rb
        in_maps.append(m)
    return in_maps


def kernel(x, omega, phase, amp_real, amp_imag, centers):
    global LAST_RESULTS
    x = np.asarray(x); omega = np.asarray(omega); phase = np.asarray(phase)
    amp_real = np.asarray(amp_real); amp_imag = np.asarray(amp_imag)
    centers = np.asarray(centers)
    assert x.shape == (B, D) and omega.shape == (K, D)

    # Safe upper bound on a = dist2/(2 sigma^2) (Cauchy-Schwarz):
    # decides whether the envelope may be approximated by R (a <= 1e-4), must
    # be kept as the linearized matmul (a <= 1e-3), or is out of regime.
    sig = (omega.astype(np.float64) ** 2).sum(1) + 1e-4
    xn = np.sqrt((x.astype(np.float64) ** 2).sum(1).max())
    cn = np.sqrt((centers.astype(np.float64) ** 2).sum(1).max())
    a_bound = (xn + cn) ** 2 / (2.0 * (sig.min() ** 2))
    drop_env = DROP_ENV and a_bound < 1e-4
    assert a_bound < 1e-3, f"envelope linearization out of regime: {a_bound=}"

    key = ("nc", drop_env)
    if key not in _CACHE:
        _CACHE[key] = _build(drop_env)
    nc = _CACHE[key]

    in_maps = _host_prep(x, omega, phase, amp_real, amp_imag, centers, drop_env)
    res = run_bass_kernel_spmd(nc, in_maps, core_ids=list(range(N_CORES)))
    LAST_RESULTS = res

    psi = np.empty((B, K), np.complex64)
    for c in range(N_CORES):
        cs = slice(c * B_SH, (c + 1) * B_SH)
        psi.real[cs] = res.results[c]["out_r"].T
        psi.imag[cs] = res.results[c]["out_i"].T
    return psi


# revision 12
# speedup vs baseline: 1.0325x; 1.0325x over previous
"""Trainium2 Bass kernel for nn_ConstantQResonantPacket (B=32768, D=512, K=1024).

psi[b,k] = exp(-dist2(x_b,c_k)/(2*sigma_k^2)) * (ar_k + i*ai_k) * exp(i*(x_b.w_k + phase_k))

Data-parallel over batch across 8 cores; on-chip layout [k partitions, b free].

Key algebra/precision moves:
  * amp -> R*e^{i*phi0}: phi0 folded into the phase offset, R into the envelope.
  * sigma_k = ||w_k||^2 + 1e-4 ~ 4600 -> dist2/(2 sigma^2) <= ~6e-5, so
    R*exp(-a) = R*(1-a) to ~1e-9 relative: the entire envelope is a LINEAR
    function of dist2 and is folded into the centers matmul accumulation:
    PSUM_c = c0 + c1*(x_sq + c_sq - 2 x.c), with c0 = R, c1 = -R/(2 sigma^2).
  * all matmuls run in fp16 at full PE rate; operands are split hi/lo on the
    host and a 3-term split (hi.hi + hi.lo + lo.hi) gives ~fp32-grade phase
    precision (~22 effective mantissa bits) at 1 cycle/row.
  * u = x @ (omega/2pi).T; range reduction via DVE magic-number round
    (w2 = round(u+phi)+M), v_neg = (w2-M)-u; then
    sin = Sin(-2pi*v_neg + 2pi*phi), |.| = Abs(-v_neg + phi),
    cos = Sin(-2pi*|.| + pi/2) -- all inside Sin's [-pi,pi] table domain.
  * real/imag = PSUM_c * cos/sin on DVE; fp32 outputs [K, B_shard];
    host transposes shards into the complex64 (B, K) result.
"""
import numpy as np

import concourse.tile as tile
from concourse import bacc, mybir
from concourse.bass_utils import run_bass_kernel_spmd
from contextlib import ExitStack

F32 = mybir.dt.float32
F16 = mybir.dt.float16
AF = mybir.ActivationFunctionType
OP = mybir.AluOpType

N_CORES = 8
B, D, K = 32768, 512, 1024
B_SH = B // N_CORES          # 4096
BT = 512                     # b tile (free dim)
KT = 128                     # k tile (partition dim)
NB = B_SH // BT              # 8
NK = K // KT                 # 8
ND = D // 128                # 4

MAGIC = float(np.float32(1.5 * 2 ** 23))
TWO_PI = float(np.float32(2.0 * np.pi))
HALF_PI = float(np.float32(np.pi / 2.0))
ENV_SCALE = 2.0 ** 24        # keeps fp16 envelope operands in normal range
ENV_DESCALE = float(np.float32(2.0 ** -24))

DROP_ENV = True   # envelope ~ R: deviation <= ~6e-5 rel, below the fp32
                  # noise floor of the reference itself (measured 6.4e-5).
                  # False computes the full linearized envelope via matmul.

_CACHE = {}
LAST_RESULTS = None


def _build(drop_env):
    nc = bacc.Bacc("TRN2", target_bir_lowering=False, debug=False,
                   num_devices=N_CORES)
    t = nc.alloc_sbuf_tensor("uconst-halfpi", [128, 1], F32)
    nc.gpsimd.memset(t.ap(), HALF_PI)
    nc.const_aps.aps[(F32, HALF_PI)] = t.ap()
    nc.all_engine_barrier()

    x_all = nc.dram_tensor("x_all", (D, 2 * B_SH), F16, kind="ExternalInput").ap()
    w_all = nc.dram_tensor("w_all", (D, 2 * K), F16, kind="ExternalInput").ap()
    small = nc.dram_tensor("small", (128, 3 * NK), F32, kind="ExternalInput").ap()
    if not drop_env:
        cTe = nc.dram_tensor("cTe", (D, K), F16, kind="ExternalInput").ap()
        lhsb = nc.dram_tensor("lhsb", (128, K), F16, kind="ExternalInput").ap()
        rhsb = nc.dram_tensor("rhsb", (128, B_SH), F16, kind="ExternalInput").ap()
    out_r = nc.dram_tensor("out_r", (K, B_SH), F32, kind="ExternalOutput").ap()
    out_i = nc.dram_tensor("out_i", (K, B_SH), F32, kind="ExternalOutput").ap()

    with tile.TileContext(nc) as tc, ExitStack() as ctx:
        par = ctx.enter_context(tc.tile_pool(name="par", bufs=1))
        xt = ctx.enter_context(tc.tile_pool(name="xt", bufs=3))
        ew = ctx.enter_context(tc.tile_pool(name="ew", bufs=4))
        ot = ctx.enter_context(tc.tile_pool(name="ot", bufs=4))
        ps = ctx.enter_context(tc.tile_pool(name="ps", bufs=6 if drop_env else 3, space="PSUM"))

        tsmall = par.tile([128, 3 * NK], F32, tag="small")
        tphi = tsmall[:, 0:NK]
        tphi2 = tsmall[:, NK:2 * NK]
        tc0 = tsmall[:, 2 * NK:3 * NK]
        tw_h, tw_l, tce = [], [], []
        tw_all, tx_all = [], []
        for d in range(ND):
            tw = par.tile([128, 2 * K], F16, tag=f"w{d}")
            tw_all.append(tw)
            tw_h.append(tw[:, 0:K])
            tw_l.append(tw[:, K:2 * K])
            if not drop_env:
                tc_ = par.tile([128, K], F16, tag=f"ce{d}")
                tce.append(tc_)
        # interleave: w chunk d, then b0's x chunk d, so k-tile 0 can start ASAP
        for d in range(ND):
            nc.sync.dma_start(tw_all[d][:], w_all[d * 128:(d + 1) * 128, :])
            xa = xt.tile([128, 2 * BT], F16, tag=f"x{d}")
            nc.sync.dma_start(xa[:], x_all[d * 128:(d + 1) * 128, 0:2 * BT])
            tx_all.append(xa)
            if d == 0:
                nc.sync.dma_start(tsmall[:], small)
        if not drop_env:
            for d in range(ND):
                nc.sync.dma_start(tce[d][:], cTe[d * 128:(d + 1) * 128, :])
            tlb = par.tile([128, K], F16, tag="lhsb")
            nc.sync.dma_start(tlb[:], lhsb)

        for b in range(NB):
            bs = slice(b * BT, (b + 1) * BT)
            if b == 0:
                txh = [xa[:, 0:BT] for xa in tx_all]
                txl = [xa[:, BT:2 * BT] for xa in tx_all]
            else:
                xa_big = xt.tile([128, ND * 2 * BT], F16, tag="xbig")
                src = x_all.rearrange("(d p) m -> p d m", p=128)
                nc.sync.dma_start(
                    xa_big[:].rearrange("p (d m) -> p d m", d=ND),
                    src[:, :, 2 * b * BT:2 * (b + 1) * BT])
                txh = [xa_big[:, d * 2 * BT:d * 2 * BT + BT] for d in range(ND)]
                txl = [xa_big[:, d * 2 * BT + BT:(d + 1) * 2 * BT] for d in range(ND)]
            if not drop_env:
                trb = xt.tile([128, BT], F16, tag="rhsb")
                nc.sync.dma_start(trb[:], rhsb[:, bs])

            for k in range(NK):
                ks = slice(k * KT, (k + 1) * KT)
                if not drop_env:
                    psc = ps.tile([KT, BT], F32, tag="psc")
                    for d in range(ND):
                        nc.tensor.matmul(psc[:], tce[d][:, ks], txh[d][:],
                                         start=(d == 0), stop=False)
                    nc.tensor.matmul(psc[:], tlb[:, ks], trb[:],
                                     start=False, stop=True)
                # u = x @ (omega/2pi).T : 3-term f32r split
                psw = ps.tile([KT, BT], F32, tag="psw")
                n_mm = 3 * ND
                i = 0
                for d in range(ND):
                    nc.tensor.matmul(psw[:], tw_h[d][:, ks], txh[d][:],
                                     start=(i == 0), stop=(i == n_mm - 1))
                    i += 1
                    nc.tensor.matmul(psw[:], tw_h[d][:, ks], txl[d][:],
                                     start=False, stop=(i == n_mm - 1))
                    i += 1
                    nc.tensor.matmul(psw[:], tw_l[d][:, ks], txh[d][:],
                                     start=False, stop=(i == n_mm - 1))
                    i += 1
                w2 = ew.tile([KT, BT], F32, tag="w2")
                nc.vector.tensor_scalar(w2[:], psw[:], tphi[:, k:k + 1], MAGIC,
                                        OP.add, OP.add)
                vneg = ew.tile([KT, BT], F32, tag="vneg")
                nc.vector.scalar_tensor_tensor(vneg[:], w2[:], MAGIC, psw[:],
                                               OP.subtract, OP.subtract)
                abst = ew.tile([KT, BT], F32, tag="abst")
                nc.scalar.activation(abst[:], vneg[:], AF.Abs,
                                     bias=tphi[:, k:k + 1], scale=-1.0)
                cost = ew.tile([KT, BT], F32, tag="cost")
                nc.scalar.activation(cost[:], abst[:], AF.Sin,
                                     bias=HALF_PI, scale=-TWO_PI)
                sint = ew.tile([KT, BT], F32, tag="sint")
                nc.scalar.activation(sint[:], vneg[:], AF.Sin,
                                     bias=tphi2[:, k:k + 1], scale=-TWO_PI)
                realt = ot.tile([KT, BT], F32, tag="realt")
                imagt = ot.tile([KT, BT], F32, tag="imagt")
                if not drop_env:
                    nc.vector.scalar_tensor_tensor(realt[:], psc[:], ENV_DESCALE,
                                                   cost[:], OP.mult, OP.mult)
                    nc.vector.scalar_tensor_tensor(imagt[:], psc[:], ENV_DESCALE,
                                                   sint[:], OP.mult, OP.mult)
                else:
                    nc.vector.tensor_scalar_mul(realt[:], cost[:], tc0[:, k:k + 1])
                    nc.vector.tensor_scalar_mul(imagt[:], sint[:], tc0[:, k:k + 1])
                nc.sync.dma_start(out_r[ks, bs], realt[:])
                nc.sync.dma_start(out_i[ks, bs], imagt[:])
    nc.compile()
    return nc


def _host_prep(x, omega, phase, amp_real, amp_imag, centers, drop_env):
    f64 = np.float64
    w64 = omega.astype(f64)
    sigma = (w64 * w64).sum(1) + 1e-4
    inv2s2 = 1.0 / (2.0 * sigma * sigma)
    R = np.hypot(amp_real.astype(f64), amp_imag.astype(f64))
    phi0 = np.arctan2(amp_imag.astype(f64), amp_real.astype(f64))
    c0 = R
    c1 = -R * inv2s2

    wT = np.ascontiguousarray((w64 / (2 * np.pi)).T.astype(np.float32))
    whT = wT.astype(np.float16)
    wlT = (wT - whT.astype(np.float32)).astype(np.float16)

    phi_v = (((phase.astype(f64) + phi0) / (2 * np.pi)) % 1.0).astype(np.float32)
    phi_t = np.ascontiguousarray(phi_v.reshape(NK, 128).T)
    phi2_t = np.ascontiguousarray(
        (phi_v.astype(f64) * (2 * np.pi)).astype(np.float32).reshape(NK, 128).T)

    x32 = x.astype(np.float32)
    xh = x32.astype(np.float16)
    xl = (x32 - xh.astype(np.float32)).astype(np.float16)
    xhT = xh.T  # [D, B]
    xlT = xl.T

    w_all = np.concatenate([whT, wlT], axis=1)  # [D, 2K]
    small = np.zeros((128, 3 * NK), np.float32)
    small[:, 0:NK] = phi_t
    small[:, NK:2 * NK] = phi2_t
    shared = dict(w_all=w_all, small=small)
    if not drop_env:
        c64 = centers.astype(f64)
        c_sq = (c64 * c64).sum(1)
        # all env products carry a 2^24 scale (fp16 range), divided out on DVE.
        bias = (c0 + c1 * c_sq) * (2.0 ** 12)        # lhs 2^12 x rhs ones 2^12
        bias_hi = bias.astype(np.float16)
        bias_lo = (bias - bias_hi.astype(np.float64)).astype(np.float16)
        cTe = np.ascontiguousarray(
            (-2.0 * ENV_SCALE * c1[:, None] * c64).T).astype(np.float32).astype(np.float16)
        lhsb = np.zeros((128, K), np.float16)
        lhsb[0] = bias_hi
        lhsb[1] = bias_lo
        lhsb[2] = (c1 * (2.0 ** 18)).astype(np.float32).astype(np.float16)
        shared.update(cTe=cTe, lhsb=lhsb)
        x_sq = ((x32.astype(f64) ** 2).sum(1) * (2.0 ** 6)).astype(np.float32).astype(np.float16)
    else:
        small[:, 2 * NK:3 * NK] = c0.astype(np.float32).reshape(NK, 128).T

    in_maps = []
    for c in range(N_CORES):
        cs = slice(c * B_SH, (c + 1) * B_SH)
        xa = np.empty((D, 2 * B_SH), np.float16)
        for b in range(NB):
            xa[:, 2 * b * BT:(2 * b + 1) * BT] = xhT[:, c * B_SH + b * BT:c * B_SH + (b + 1) * BT]
            xa[:, (2 * b + 1) * BT:2 * (b + 1) * BT] = xlT[:, c * B_SH + b * BT:c * B_SH + (b + 1) * BT]
        m = dict(shared, x_all=xa)
        if not drop_env:
            rb = np.zeros((128, B_SH), np.float16)
            rb[0] = np.float16(2.0 ** 12)
            rb[1] = np.float16(2.0 ** 12)
            rb[2] = x_sq[cs]
            m["rhsb"] = rb
        in_maps.append(m)
    return in_maps


def kernel(x, omega, phase, amp_real, amp_imag, centers):
    global LAST_RESULTS
    x = np.asarray(x); omega = np.asarray(omega); phase = np.asarray(phase)
    amp_real = np.asarray(amp_real); amp_imag = np.asarray(amp_imag)
    centers = np.asarray(centers)
    assert x.shape == (B, D) and omega.shape == (K, D)

    # Safe upper bound on a = dist2/(2 sigma^2) (Cauchy-Schwarz):
    # decides whether the envelope may be approximated by R (a <= 1e-4), must
    # be kept as the linearized matmul (a <= 1e-3), or is out of regime.
    sig = (omega.astype(np.float64) ** 2).sum(1) + 1e-4
    xn = np.sqrt((x.astype(np.float64) ** 2).sum(1).max())
    cn = np.sqrt((centers.astype(np.float64) ** 2).sum(1).max())
    a_bound = (xn + cn) ** 2 / (2.0 * (sig.min() ** 2))
    drop_env = DROP_ENV and a_bound < 1e-4
    assert a_bound < 1e-3, f"envelope linearization out of regime: {a_bound=}"

    key = ("nc", drop_env)
    if key not in _CACHE:
        _CACHE[key] = _build(drop_env)
    nc = _CACHE[key]

    in_maps = _host_prep(x, omega, phase, amp_real, amp_imag, centers, drop_env)
    res = run_bass_kernel_spmd(nc, in_maps, core_ids=list(range(N_CORES)))
    LAST_RESULTS = res

    psi = np.empty((B, K), np.complex64)
    for c in range(N_CORES):
        cs = slice(c * B_SH, (c + 1) * B_SH)
        psi.real[cs] = res.results[c]["out_r"].T
        psi.imag[cs] = res.results[c]["out_i"].T
    return psi
